# revision 15
# baseline (speedup 1.0000x reference)
"""DIN attention Bass kernel for Trainium2, 8-core data-parallel.

Design (per core, BL=256 rows, 8 chunks of 32 rows):
- Token compaction: only unmasked tokens are shipped (max count 127 <= 128).
  Rows are globally sorted by token count into 8 bands of 256; band ci is
  chunk index ci on every core, so all cores share per-chunk width W_c
  (multiple of 8), keeping the SPMD program identical across cores.
- W1 fused as ONE fp8 DoubleRow matmul per row: plane0 = K + a_b,
  plane1 = q_b*K + v_b where [a_b; v_b] is the min-norm solution of
  [W1bc.T | W1d.T] [a; v] = qb (qb = q(W1a+W1c)+b1).  The relu1 bias
  vanishes; relu1 becomes a big batched op.
- Layout per chunk: 4 ps1 tiles (2 PSUM banks each) hold 8 rows' h1-pre;
  relu1 -> h1 [128, 8, W] bf16.  W2 packs 8 rows per ps2 bank
  (2 partition-halves x 4 col-quarters); relu2 [128, 2, 4W] -> h2.
  W3 (block-diag w3s [128, 2]) -> scores at ps3 rows {32j, 32j+1}.
  exp as one [98, 4W] op; 4 PE transposes -> wps [W, 4, 98] bf16.
- Weighted sum: per-row N=1 matmul (ktm stationary, free ldweights);
  U via per-row mask-column stationary (N=1).  sums+U DMA'd out
  unnormalized; host divides (and unsorts).
"""

import numpy as np

B, S, E = 2048, 200, 128
H1, H2 = 128, 64
NCORES = 8
BL = B // NCORES          # 256
CHUNK = 32
NCHUNK = BL // CHUNK      # 8

_prog_cache = {}


def _build_program(widths):
    import concourse.bass as bass
    import concourse.mybir as mybir
    import concourse.tile as tile
    from concourse import bacc
    from concourse.masks import make_identity
    from contextlib import ExitStack

    f32 = mybir.dt.float32
    bf16 = mybir.dt.bfloat16
    fp8 = mybir.dt.float8e4
    AF = mybir.ActivationFunctionType
    ALU = mybir.AluOpType
    DR = mybir.MatmulPerfMode.DoubleRow

    nc = bacc.Bacc(None, target_bir_lowering=False, debug=False)

    rhs_d = [nc.declare_dram_parameter(f"rhs{ci}", [E, CHUNK, 2, widths[ci]],
                                       fp8, False) for ci in range(NCHUNK)]
    ktm_d = [nc.declare_dram_parameter(f"ktm{ci}", [widths[ci], CHUNK, E],
                                       bf16, False) for ci in range(NCHUNK)]
    mskt_d = nc.declare_dram_parameter("mskt", [128, NCHUNK, CHUNK], bf16, False)
    w1dr_d = nc.declare_dram_parameter("w1dr", [E, 2, H1], fp8, False)
    w2_d = nc.declare_dram_parameter("w2", [H1, H2], bf16, False)
    w3s_d = nc.declare_dram_parameter("w3s", [2 * H2, 32], bf16, False)
    b2s_d = nc.declare_dram_parameter("b2s", [2 * H2, 1], f32, False)
    b3v_d = nc.declare_dram_parameter("b3v", [1, 1], f32, False)
    out_d = nc.declare_dram_parameter("out", [E, NCHUNK * 2 * CHUNK], f32, True)

    with tile.TileContext(nc) as tc, ExitStack() as ctx:
        const = ctx.enter_context(tc.tile_pool(name="const", bufs=1))
        kpool = ctx.enter_context(tc.tile_pool(name="keys", bufs=1))
        work = ctx.enter_context(tc.tile_pool(name="work", bufs=3))
        spool = ctx.enter_context(tc.tile_pool(name="smax", bufs=2))
        ps1p = ctx.enter_context(tc.tile_pool(name="ps1", bufs=2, space="PSUM"))
        ps2p = ctx.enter_context(tc.tile_pool(name="ps2", bufs=1, space="PSUM"))
        ps3p = ctx.enter_context(tc.tile_pool(name="ps3", bufs=1, space="PSUM"))
        wpsp = ctx.enter_context(tc.tile_pool(name="wps", bufs=1, space="PSUM"))

        w1dr = const.tile([E, 2, H1], fp8)
        nc.scalar.dma_start(w1dr, w1dr_d[:])
        w2c = const.tile([H1, H2], bf16)
        nc.scalar.dma_start(w2c, w2_d[:])
        w3s = const.tile([2 * H2, 32], bf16)
        nc.scalar.dma_start(w3s, w3s_d[:])
        b2s = const.tile([2 * H2, 1], f32)
        nc.scalar.dma_start(b2s, b2s_d[:])
        b3t = const.tile([128, 1], f32)
        nc.scalar.dma_start(b3t, b3v_d[:].to_broadcast((128, 1)))
        mskt = const.tile([128, NCHUNK, CHUNK], bf16)
        nc.scalar.dma_start(mskt, mskt_d[:])
        ident_bf = const.tile([128, 128], bf16)
        make_identity(nc, ident_bf)
        outbuf = const.tile([E, NCHUNK, 2 * CHUNK], f32)
        nc.vector.memset(outbuf, 0.0)

        for ci in range(NCHUNK):
            W = widths[ci]
            # ---- input DMAs: alternate queues per chunk for balance ----
            rhs8 = kpool.tile([E, CHUNK, 2, W], fp8, tag=f"rhs{ci}")
            ktm = kpool.tile([128, CHUNK, E], bf16, tag=f"ktm{ci}")
            if ci % 2 == 0:
                nc.sync.dma_start(rhs8, rhs_d[ci][:])
                nc.gpsimd.dma_start(ktm[0:W], ktm_d[ci][:])
            else:
                nc.gpsimd.dma_start(rhs8, rhs_d[ci][:])
                nc.sync.dma_start(ktm[0:W], ktm_d[ci][:])

            ps3 = ps3p.tile([128, 512], f32, tag="ps3")
            pso = ps3[:, 448:512]
            h1s = []
            for t in range(4):        # 8 rows per t
                ps1 = ps1p.tile([128, 2, 512], f32, tag="ps1")
                for u in range(8):
                    slot = 8 * t + u
                    nc.tensor.matmul(
                        ps1[:, u // 4, (u % 4) * W:(u % 4 + 1) * W],
                        w1dr, rhs8[:, slot, :, :],
                        start=True, stop=True, perf_mode=DR,
                        skip_group_check=True)
                h1 = work.tile([128, 8, W], bf16, tag="h1")
                h1s.append(h1)
                # relu1: two [128, 4, W] half-ops on ACT + DVE in parallel
                for hf in range(2):
                    src = ps1[:, hf, 0:4 * W].rearrange("p (c b) -> p c b",
                                                        b=W)
                    dst = h1[:, 4 * hf:4 * hf + 4, :]
                    if (t + hf) % 2 == 0:
                        nc.scalar.activation(dst, src, AF.Relu)
                    else:
                        nc.vector.tensor_scalar(dst, src, 0.0, None, ALU.max)
                # W2: 8 rows into one ps2 bank-half
                if t % 2 == 0:
                    ps2 = ps2p.tile([128, 2, 512], f32, tag="ps2")
                for u in range(8):
                    qq, r = u // 2, u % 2
                    nc.tensor.matmul(
                        ps2[64 * r:64 * r + 64, t % 2, qq * W:(qq + 1) * W],
                        w2c, h1[:, u, :], start=True, stop=True,
                        tile_position=(0, 64 * r), skip_group_check=True)
                if t % 2 == 1:
                    h2 = work.tile([128, 2, 4, W], bf16, tag="h2")
                    for tt in range(2):
                        src2 = ps2[:, tt, 0:4 * W].rearrange(
                            "p (c b) -> p c b", b=W)
                        dst2 = h2[:, tt]
                        if (t // 2 + tt) % 2 == 0:
                            nc.vector.tensor_scalar(dst2, src2, b2s[:, 0:1],
                                                    0.0, ALU.add, ALU.max)
                        else:
                            nc.scalar.activation(dst2, src2, AF.Relu,
                                                 bias=b2s[:, 0:1])
                        j = (t - 1) + tt
                        nc.tensor.matmul(
                            ps3[32 * j:32 * j + 32, 0:4 * W], w3s,
                            h2[:, tt, :, :].rearrange("p a b -> p (a b)"),
                            start=True, stop=True, tile_position=(0, 32 * j),
                            skip_group_check=True)
            # ---- exp over all 32 rows' scores ----
            u_sp = spool.tile([98, 4, W], bf16, tag="usp")
            for eh in range(2):
                nc.scalar.activation(
                    u_sp[:, 2 * eh:2 * eh + 2, :],
                    ps3[0:98, 2 * eh * W:(2 * eh + 2) * W].rearrange(
                        "p (a b) -> p a b", b=W),
                    AF.Exp, bias=b3t[0:98, 0:1])
            wps = wpsp.tile([128, 4, 128], bf16, tag="wps")
            for qq in range(4):
                nc.tensor.transpose(wps[0:W, qq, 0:98], u_sp[:, qq, 0:W],
                                    ident_bf[0:98, 0:98])
            # evac live transpose columns (rows 32j+r) to SBUF for the PE
            wcol = spool.tile([128, 4, 4, 2], bf16, tag="wcol")
            wv = wps.rearrange("p q (j x) -> p q j x", x=32)
            nc.vector.tensor_copy(wcol[0:W], wv[0:W, :, :, 0:2])
            # ---- weighted sums + U ----
            for slot in range(CHUNK):
                j, qq, r = slot // 8, (slot % 8) // 2, slot % 2
                wc = wcol[0:W, qq, j, r:r + 1]
                nc.tensor.matmul(pso[:, slot:slot + 1], ktm[0:W, slot, :], wc,
                                 start=True, stop=True, skip_group_check=True)
                nc.tensor.matmul(pso[0:1, CHUNK + slot:CHUNK + slot + 1],
                                 mskt[0:W, ci, slot:slot + 1], wc,
                                 start=True, stop=True, skip_group_check=True)
            nc.vector.tensor_copy(outbuf[:, ci, 0:CHUNK], pso[:, 0:CHUNK])
            nc.vector.tensor_copy(outbuf[0:1, ci, CHUNK:2 * CHUNK],
                                  pso[0:1, CHUNK:2 * CHUNK])
        nc.scalar.dma_start(out_d[:], outbuf.rearrange("p a b -> p (a b)"))
    nc.compile()
    return nc


def _host_prep(querys, keys, W1, b1, W2, b2, W3, b3, mask):
    import ml_dtypes
    bf = ml_dtypes.bfloat16
    f8 = ml_dtypes.float8_e4m3
    q = np.ascontiguousarray(querys[:, 0, :], dtype=np.float32)   # [B, E]
    W1a, W1b, W1c, W1d = W1[0:128], W1[128:256], W1[256:384], W1[384:512]
    W1bc = (W1b - W1c).astype(np.float32)
    qb = q @ (W1a + W1c) + b1                                      # [B, H1]
    # min-norm absorption of qb into the two DoubleRow planes
    A = np.concatenate([W1bc.T, W1d.T], axis=1)                    # [128, 256]
    av = (A.T @ np.linalg.solve(A @ A.T, qb.T)).T                  # [B, 256]
    a_b, v_b = av[:, :128], av[:, 128:]

    counts = mask.sum(axis=1).astype(np.int64)                     # [B]
    assert counts.max() <= 128, f"token count {counts.max()} > 128 unsupported"
    order = np.argsort(counts, kind="stable")                      # ascending
    widths = []
    for ci in range(NCHUNK):
        band = order[ci * NCORES * CHUNK:(ci + 1) * NCORES * CHUNK]
        widths.append(max(8, int(-(-counts[band].max() // 8) * 8)))
    widths = tuple(int(w) for w in widths)

    # row assignment: core c, chunk ci, slot s -> order[ci*256 + c*32 + s]
    assign = order.reshape(NCHUNK, NCORES, CHUNK)

    rhs_arrs = [[] for _ in range(NCORES)]
    ktm_arrs = [[] for _ in range(NCORES)]
    mskt_arr = np.zeros((NCORES, 128, NCHUNK, CHUNK), np.float32)
    for ci in range(NCHUNK):
        W = widths[ci]
        for c in range(NCORES):
            rows = assign[ci, c]                                   # [32]
            Kg = np.zeros((CHUNK, W, E), np.float32)
            for s_i, r_i in enumerate(rows):
                toks = np.nonzero(mask[r_i])[0]
                Kg[s_i, :len(toks)] = keys[r_i, toks]
                mskt_arr[c, :len(toks), ci, s_i] = 1.0
            p0 = Kg + a_b[rows][:, None, :]
            p1 = Kg * q[rows][:, None, :] + v_b[rows][:, None, :]
            rhs = np.stack([p0, p1], axis=1)                       # [32,2,W,E]
            rhs_arrs[c].append(np.ascontiguousarray(
                rhs.transpose(3, 0, 1, 2)).astype(f8))             # [E,32,2,W]
            ktm_arrs[c].append(np.ascontiguousarray(
                Kg.transpose(1, 0, 2)).astype(bf))                 # [W,32,E]

    w1dr = np.ascontiguousarray(
        np.stack([W1bc, W1d], axis=1)).astype(f8)                  # [E,2,H1]
    w3s = np.zeros((2 * H2, 32), bf)
    w3s[0:H2, 0] = W3[:, 0].astype(bf)
    w3s[H2:, 1] = W3[:, 0].astype(bf)
    b2s = np.concatenate([b2, b2]).reshape(2 * H2, 1).astype(np.float32)
    return dict(widths=widths, assign=assign, rhs=rhs_arrs, ktm=ktm_arrs,
                mskt=mskt_arr.astype(bf), w1dr=w1dr,
                w2=W2.astype(bf), w3s=w3s, b2s=b2s,
                b3v=np.asarray(b3, np.float32).reshape(1, 1))


def kernel(querys, keys, W1, b1, W2, b2, W3, b3, mask):
    from concourse.bass_utils import run_bass_kernel_spmd

    querys = np.asarray(querys, dtype=np.float32)
    keys = np.asarray(keys, dtype=np.float32)
    W1 = np.asarray(W1, dtype=np.float32)
    b1 = np.asarray(b1, dtype=np.float32)
    W2 = np.asarray(W2, dtype=np.float32)
    b2 = np.asarray(b2, dtype=np.float32)
    W3 = np.asarray(W3, dtype=np.float32)
    b3 = np.asarray(b3, dtype=np.float32)
    mask = np.asarray(mask)
    hp = _host_prep(querys, keys, W1, b1, W2, b2, W3, b3, mask)

    widths = hp["widths"]
    if widths not in _prog_cache:
        _prog_cache[widths] = _build_program(widths)
    prog = _prog_cache[widths]

    in_maps = []
    for c in range(NCORES):
        m = {f"rhs{ci}": hp["rhs"][c][ci] for ci in range(NCHUNK)}
        m.update({f"ktm{ci}": hp["ktm"][c][ci] for ci in range(NCHUNK)})
        m.update({"mskt": hp["mskt"][c], "w1dr": hp["w1dr"], "w2": hp["w2"],
                  "w3s": hp["w3s"], "b2s": hp["b2s"], "b3v": hp["b3v"]})
        in_maps.append(m)

    res = run_bass_kernel_spmd(prog, in_maps, list(range(NCORES)))
    out = np.empty((B, E), np.float32)
    assign = hp["assign"]
    for c in range(NCORES):
        o = res.results[c]["out"].reshape(E, NCHUNK, 2 * CHUNK)
        for ci in range(NCHUNK):
            sums = o[:, ci, 0:CHUNK]                  # [E, 32]
            U = o[0, ci, CHUNK:2 * CHUNK]             # [32]
            out[assign[ci, c]] = (sums / U[None, :]).T
    return out


# revision 16
# speedup vs baseline: 1.0019x; 1.0019x over previous
"""DIN attention Bass kernel for Trainium2, 8-core data-parallel.

Design (per core, BL=256 rows, 8 chunks of 32 rows):
- Token compaction: only unmasked tokens are shipped (max count 127 <= 128).
  Rows are globally sorted by token count into 8 bands of 256; band ci is
  chunk index ci on every core, so all cores share per-chunk width W_c
  (multiple of 8), keeping the SPMD program identical across cores.
- W1 fused as ONE fp8 DoubleRow matmul per row: plane0 = K + a_b,
  plane1 = q_b*K + v_b where [a_b; v_b] is the min-norm solution of
  [W1bc.T | W1d.T] [a; v] = qb (qb = q(W1a+W1c)+b1).  The relu1 bias
  vanishes; relu1 becomes a big batched op.
- Layout per chunk: 4 ps1 tiles (2 PSUM banks each) hold 8 rows' h1-pre;
  relu1 -> h1 [128, 8, W] bf16.  W2 packs 8 rows per ps2 bank
  (2 partition-halves x 4 col-quarters); relu2 [128, 2, 4W] -> h2.
  W3 (block-diag w3s [128, 2]) -> scores at ps3 rows {32j, 32j+1}.
  exp as one [98, 4W] op; 4 PE transposes -> wps [W, 4, 98] bf16.
- Weighted sum: per-row N=1 matmul (ktm stationary, free ldweights);
  U via per-row mask-column stationary (N=1).  sums+U DMA'd out
  unnormalized; host divides (and unsorts).
"""

import numpy as np

B, S, E = 2048, 200, 128
H1, H2 = 128, 64
NCORES = 8
BL = B // NCORES          # 256
CHUNK = 32
NCHUNK = BL // CHUNK      # 8

_prog_cache = {}


def _build_program(widths):
    import concourse.bass as bass
    import concourse.mybir as mybir
    import concourse.tile as tile
    from concourse import bacc
    from concourse.masks import make_identity
    from contextlib import ExitStack

    f32 = mybir.dt.float32
    bf16 = mybir.dt.bfloat16
    fp8 = mybir.dt.float8e4
    AF = mybir.ActivationFunctionType
    ALU = mybir.AluOpType
    DR = mybir.MatmulPerfMode.DoubleRow

    nc = bacc.Bacc(None, target_bir_lowering=False, debug=False)

    rhs_d = [nc.declare_dram_parameter(f"rhs{ci}", [E, CHUNK, 2, widths[ci]],
                                       fp8, False) for ci in range(NCHUNK)]
    ktm_d = [nc.declare_dram_parameter(f"ktm{ci}", [widths[ci], CHUNK, E],
                                       bf16, False) for ci in range(NCHUNK)]
    mskt_d = nc.declare_dram_parameter("mskt", [128, NCHUNK, CHUNK], bf16, False)
    w1dr_d = nc.declare_dram_parameter("w1dr", [E, 2, H1], fp8, False)
    w2_d = nc.declare_dram_parameter("w2", [H1, H2], bf16, False)
    w3s_d = nc.declare_dram_parameter("w3s", [2 * H2, 32], bf16, False)
    b2s_d = nc.declare_dram_parameter("b2s", [2 * H2, 1], f32, False)
    b3v_d = nc.declare_dram_parameter("b3v", [1, 1], f32, False)
    out_d = nc.declare_dram_parameter("out", [E, NCHUNK * 2 * CHUNK], f32, True)

    with tile.TileContext(nc) as tc, ExitStack() as ctx:
        const = ctx.enter_context(tc.tile_pool(name="const", bufs=1))
        kpool = ctx.enter_context(tc.tile_pool(name="keys", bufs=1))
        work = ctx.enter_context(tc.tile_pool(name="work", bufs=3))
        spool = ctx.enter_context(tc.tile_pool(name="smax", bufs=2))
        ps1p = ctx.enter_context(tc.tile_pool(name="ps1", bufs=2, space="PSUM"))
        ps2p = ctx.enter_context(tc.tile_pool(name="ps2", bufs=1, space="PSUM"))
        ps3p = ctx.enter_context(tc.tile_pool(name="ps3", bufs=1, space="PSUM"))
        wpsp = ctx.enter_context(tc.tile_pool(name="wps", bufs=1, space="PSUM"))

        w1dr = const.tile([E, 2, H1], fp8)
        nc.scalar.dma_start(w1dr, w1dr_d[:])
        w2c = const.tile([H1, H2], bf16)
        nc.scalar.dma_start(w2c, w2_d[:])
        w3s = const.tile([2 * H2, 32], bf16)
        nc.scalar.dma_start(w3s, w3s_d[:])
        b2s = const.tile([2 * H2, 1], f32)
        nc.scalar.dma_start(b2s, b2s_d[:])
        b3t = const.tile([128, 1], f32)
        nc.scalar.dma_start(b3t, b3v_d[:].to_broadcast((128, 1)))
        mskt = const.tile([128, NCHUNK, CHUNK], bf16)
        nc.scalar.dma_start(mskt, mskt_d[:])
        ident_bf = const.tile([128, 128], bf16)
        make_identity(nc, ident_bf)
        outbuf = const.tile([E, NCHUNK, 2 * CHUNK], f32)
        nc.vector.memset(outbuf, 0.0)

        for ci in range(NCHUNK):
            W = widths[ci]
            # ---- input DMAs: alternate queues per chunk for balance ----
            rhs8 = kpool.tile([E, CHUNK, 2, W], fp8, tag=f"rhs{ci}")
            ktm = kpool.tile([128, CHUNK, E], bf16, tag=f"ktm{ci}")
            if ci % 2 == 0:
                nc.sync.dma_start(rhs8, rhs_d[ci][:])
                nc.gpsimd.dma_start(ktm[0:W], ktm_d[ci][:])
            else:
                nc.gpsimd.dma_start(rhs8, rhs_d[ci][:])
                nc.sync.dma_start(ktm[0:W], ktm_d[ci][:])

            ps3 = ps3p.tile([128, 512], f32, tag="ps3")
            pso = ps3[:, 448:512]
            h1s = []
            for t in range(4):        # 8 rows per t
                h1 = work.tile([128, 8, W], bf16, tag="h1")
                h1s.append(h1)
                # two independent 1-bank ps1 tiles; relu halves ACT/DVE
                for hf in range(2):
                    ps1 = ps1p.tile([128, 512], f32, tag=f"ps1{hf}")
                    for u4 in range(4):
                        slot = 8 * t + 4 * hf + u4
                        nc.tensor.matmul(
                            ps1[:, u4 * W:(u4 + 1) * W],
                            w1dr, rhs8[:, slot, :, :],
                            start=True, stop=True, perf_mode=DR,
                            skip_group_check=True)
                    src = ps1[:, 0:4 * W].rearrange("p (c b) -> p c b", b=W)
                    dst = h1[:, 4 * hf:4 * hf + 4, :]
                    if (t + hf) % 2 == 0:
                        nc.scalar.activation(dst, src, AF.Relu)
                    else:
                        nc.vector.tensor_scalar(dst, src, 0.0, None, ALU.max)
                # W2: 8 rows into one ps2 bank-half
                if t % 2 == 0:
                    ps2 = ps2p.tile([128, 2, 512], f32, tag="ps2")
                for u in range(8):
                    qq, r = u // 2, u % 2
                    nc.tensor.matmul(
                        ps2[64 * r:64 * r + 64, t % 2, qq * W:(qq + 1) * W],
                        w2c, h1[:, u, :], start=True, stop=True,
                        tile_position=(0, 64 * r), skip_group_check=True)
                if t % 2 == 1:
                    h2 = work.tile([128, 2, 4, W], bf16, tag="h2")
                    for tt in range(2):
                        src2 = ps2[:, tt, 0:4 * W].rearrange(
                            "p (c b) -> p c b", b=W)
                        dst2 = h2[:, tt]
                        if (t // 2 + tt) % 2 == 0:
                            nc.vector.tensor_scalar(dst2, src2, b2s[:, 0:1],
                                                    0.0, ALU.add, ALU.max)
                        else:
                            nc.scalar.activation(dst2, src2, AF.Relu,
                                                 bias=b2s[:, 0:1])
                        j = (t - 1) + tt
                        nc.tensor.matmul(
                            ps3[32 * j:32 * j + 32, 0:4 * W], w3s,
                            h2[:, tt, :, :].rearrange("p a b -> p (a b)"),
                            start=True, stop=True, tile_position=(0, 32 * j),
                            skip_group_check=True)
            # ---- exp over all 32 rows' scores ----
            u_sp = spool.tile([98, 4, W], bf16, tag="usp")
            for eh in range(2):
                nc.scalar.activation(
                    u_sp[:, 2 * eh:2 * eh + 2, :],
                    ps3[0:98, 2 * eh * W:(2 * eh + 2) * W].rearrange(
                        "p (a b) -> p a b", b=W),
                    AF.Exp, bias=b3t[0:98, 0:1])
            wps = wpsp.tile([128, 4, 128], bf16, tag="wps")
            for qq in range(4):
                nc.tensor.transpose(wps[0:W, qq, 0:98], u_sp[:, qq, 0:W],
                                    ident_bf[0:98, 0:98])
            # evac live transpose columns (rows 32j+r) to SBUF for the PE
            wcol = spool.tile([128, 4, 4, 2], bf16, tag="wcol")
            wv = wps.rearrange("p q (j x) -> p q j x", x=32)
            nc.vector.tensor_copy(wcol[0:W], wv[0:W, :, :, 0:2])
            # ---- weighted sums + U ----
            for slot in range(CHUNK):
                j, qq, r = slot // 8, (slot % 8) // 2, slot % 2
                wc = wcol[0:W, qq, j, r:r + 1]
                nc.tensor.matmul(pso[:, slot:slot + 1], ktm[0:W, slot, :], wc,
                                 start=True, stop=True, skip_group_check=True)
                nc.tensor.matmul(pso[0:1, CHUNK + slot:CHUNK + slot + 1],
                                 mskt[0:W, ci, slot:slot + 1], wc,
                                 start=True, stop=True, skip_group_check=True)
            nc.vector.tensor_copy(outbuf[:, ci, 0:CHUNK], pso[:, 0:CHUNK])
            nc.vector.tensor_copy(outbuf[0:1, ci, CHUNK:2 * CHUNK],
                                  pso[0:1, CHUNK:2 * CHUNK])
        nc.scalar.dma_start(out_d[:], outbuf.rearrange("p a b -> p (a b)"))
    nc.compile()
    return nc


def _host_prep(querys, keys, W1, b1, W2, b2, W3, b3, mask):
    import ml_dtypes
    bf = ml_dtypes.bfloat16
    f8 = ml_dtypes.float8_e4m3
    q = np.ascontiguousarray(querys[:, 0, :], dtype=np.float32)   # [B, E]
    W1a, W1b, W1c, W1d = W1[0:128], W1[128:256], W1[256:384], W1[384:512]
    W1bc = (W1b - W1c).astype(np.float32)
    qb = q @ (W1a + W1c) + b1                                      # [B, H1]
    # min-norm absorption of qb into the two DoubleRow planes
    A = np.concatenate([W1bc.T, W1d.T], axis=1)                    # [128, 256]
    av = (A.T @ np.linalg.solve(A @ A.T, qb.T)).T                  # [B, 256]
    a_b, v_b = av[:, :128], av[:, 128:]

    counts = mask.sum(axis=1).astype(np.int64)                     # [B]
    assert counts.max() <= 128, f"token count {counts.max()} > 128 unsupported"
    order = np.argsort(counts, kind="stable")                      # ascending
    widths = []
    for ci in range(NCHUNK):
        band = order[ci * NCORES * CHUNK:(ci + 1) * NCORES * CHUNK]
        widths.append(max(8, int(-(-counts[band].max() // 8) * 8)))
    widths = tuple(int(w) for w in widths)

    # row assignment: core c, chunk ci, slot s -> order[ci*256 + c*32 + s]
    assign = order.reshape(NCHUNK, NCORES, CHUNK)

    rhs_arrs = [[] for _ in range(NCORES)]
    ktm_arrs = [[] for _ in range(NCORES)]
    mskt_arr = np.zeros((NCORES, 128, NCHUNK, CHUNK), np.float32)
    for ci in range(NCHUNK):
        W = widths[ci]
        for c in range(NCORES):
            rows = assign[ci, c]                                   # [32]
            Kg = np.zeros((CHUNK, W, E), np.float32)
            for s_i, r_i in enumerate(rows):
                toks = np.nonzero(mask[r_i])[0]
                Kg[s_i, :len(toks)] = keys[r_i, toks]
                mskt_arr[c, :len(toks), ci, s_i] = 1.0
            p0 = Kg + a_b[rows][:, None, :]
            p1 = Kg * q[rows][:, None, :] + v_b[rows][:, None, :]
            rhs = np.stack([p0, p1], axis=1)                       # [32,2,W,E]
            rhs_arrs[c].append(np.ascontiguousarray(
                rhs.transpose(3, 0, 1, 2)).astype(f8))             # [E,32,2,W]
            ktm_arrs[c].append(np.ascontiguousarray(
                Kg.transpose(1, 0, 2)).astype(bf))                 # [W,32,E]

    w1dr = np.ascontiguousarray(
        np.stack([W1bc, W1d], axis=1)).astype(f8)                  # [E,2,H1]
    w3s = np.zeros((2 * H2, 32), bf)
    w3s[0:H2, 0] = W3[:, 0].astype(bf)
    w3s[H2:, 1] = W3[:, 0].astype(bf)
    b2s = np.concatenate([b2, b2]).reshape(2 * H2, 1).astype(np.float32)
    return dict(widths=widths, assign=assign, rhs=rhs_arrs, ktm=ktm_arrs,
                mskt=mskt_arr.astype(bf), w1dr=w1dr,
                w2=W2.astype(bf), w3s=w3s, b2s=b2s,
                b3v=np.asarray(b3, np.float32).reshape(1, 1))


def kernel(querys, keys, W1, b1, W2, b2, W3, b3, mask):
    from concourse.bass_utils import run_bass_kernel_spmd

    querys = np.asarray(querys, dtype=np.float32)
    keys = np.asarray(keys, dtype=np.float32)
    W1 = np.asarray(W1, dtype=np.float32)
    b1 = np.asarray(b1, dtype=np.float32)
    W2 = np.asarray(W2, dtype=np.float32)
    b2 = np.asarray(b2, dtype=np.float32)
    W3 = np.asarray(W3, dtype=np.float32)
    b3 = np.asarray(b3, dtype=np.float32)
    mask = np.asarray(mask)
    hp = _host_prep(querys, keys, W1, b1, W2, b2, W3, b3, mask)

    widths = hp["widths"]
    if widths not in _prog_cache:
        _prog_cache[widths] = _build_program(widths)
    prog = _prog_cache[widths]

    in_maps = []
    for c in range(NCORES):
        m = {f"rhs{ci}": hp["rhs"][c][ci] for ci in range(NCHUNK)}
        m.update({f"ktm{ci}": hp["ktm"][c][ci] for ci in range(NCHUNK)})
        m.update({"mskt": hp["mskt"][c], "w1dr": hp["w1dr"], "w2": hp["w2"],
                  "w3s": hp["w3s"], "b2s": hp["b2s"], "b3v": hp["b3v"]})
        in_maps.append(m)

    res = run_bass_kernel_spmd(prog, in_maps, list(range(NCORES)))
    out = np.empty((B, E), np.float32)
    assign = hp["assign"]
    for c in range(NCORES):
        o = res.results[c]["out"].reshape(E, NCHUNK, 2 * CHUNK)
        for ci in range(NCHUNK):
            sums = o[:, ci, 0:CHUNK]                  # [E, 32]
            U = o[0, ci, CHUNK:2 * CHUNK]             # [32]
            out[assign[ci, c]] = (sums / U[None, :]).T
    return out


# revision 17
# speedup vs baseline: 1.0577x; 1.0556x over previous
"""DIN attention Bass kernel for Trainium2, 8-core data-parallel.

Design (per core, BL=256 rows, 8 chunks of 32 rows):
- Token compaction: only unmasked tokens are shipped (max count 127 <= 128).
  Rows are globally sorted by token count into 8 bands of 256; band ci is
  chunk index ci on every core, so all cores share per-chunk width W_c
  (multiple of 8), keeping the SPMD program identical across cores.
- W1 fused as ONE fp8 DoubleRow matmul per row: plane0 = K + a_b,
  plane1 = q_b*K + v_b where [a_b; v_b] is the min-norm solution of
  [W1bc.T | W1d.T] [a; v] = qb (qb = q(W1a+W1c)+b1).  The relu1 bias
  vanishes; relu1 becomes a big batched op.
- Layout per chunk: 4 ps1 tiles (2 PSUM banks each) hold 8 rows' h1-pre;
  relu1 -> h1 [128, 8, W] bf16.  W2 packs 8 rows per ps2 bank
  (2 partition-halves x 4 col-quarters); relu2 [128, 2, 4W] -> h2.
  W3 (block-diag w3s [128, 2]) -> scores at ps3 rows {32j, 32j+1}.
  exp as one [98, 4W] op; 4 PE transposes -> wps [W, 4, 98] bf16.
- Weighted sum: per-row N=1 matmul (ktm stationary, free ldweights);
  U via per-row mask-column stationary (N=1).  sums+U DMA'd out
  unnormalized; host divides (and unsorts).
"""

import numpy as np

B, S, E = 2048, 200, 128
H1, H2 = 128, 64
NCORES = 8
BL = B // NCORES          # 256
CHUNK = 32
NCHUNK = BL // CHUNK      # 8

_prog_cache = {}


def _build_program(widths):
    import concourse.bass as bass
    import concourse.mybir as mybir
    import concourse.tile as tile
    from concourse import bacc
    from concourse.masks import make_identity
    from contextlib import ExitStack

    f32 = mybir.dt.float32
    bf16 = mybir.dt.bfloat16
    fp8 = mybir.dt.float8e4
    AF = mybir.ActivationFunctionType
    ALU = mybir.AluOpType
    DR = mybir.MatmulPerfMode.DoubleRow

    nc = bacc.Bacc(None, target_bir_lowering=False, debug=False)

    rhs_d = [nc.declare_dram_parameter(f"rhs{ci}", [E, CHUNK, 2, widths[ci]],
                                       fp8, False) for ci in range(NCHUNK)]
    ktm_d = [nc.declare_dram_parameter(f"ktm{ci}", [widths[ci], CHUNK, E],
                                       bf16, False) for ci in range(NCHUNK)]
    mskt_d = nc.declare_dram_parameter("mskt", [128, NCHUNK, CHUNK], bf16, False)
    w1dr_d = nc.declare_dram_parameter("w1dr", [E, 2, H1], fp8, False)
    w2_d = nc.declare_dram_parameter("w2", [H1, H2], bf16, False)
    w3s_d = nc.declare_dram_parameter("w3s", [2 * H2, 32], bf16, False)
    b2s_d = nc.declare_dram_parameter("b2s", [2 * H2, 1], f32, False)
    b3v_d = nc.declare_dram_parameter("b3v", [1, 1], f32, False)
    out_d = nc.declare_dram_parameter("out", [E, NCHUNK * 2 * CHUNK], f32, True)

    with tile.TileContext(nc) as tc, ExitStack() as ctx:
        const = ctx.enter_context(tc.tile_pool(name="const", bufs=1))
        kpool = ctx.enter_context(tc.tile_pool(name="keys", bufs=1))
        work = ctx.enter_context(tc.tile_pool(name="work", bufs=3))
        spool = ctx.enter_context(tc.tile_pool(name="smax", bufs=2))
        ps1p = ctx.enter_context(tc.tile_pool(name="ps1", bufs=2, space="PSUM"))
        ps2p = ctx.enter_context(tc.tile_pool(name="ps2", bufs=1, space="PSUM"))
        ps3p = ctx.enter_context(tc.tile_pool(name="ps3", bufs=1, space="PSUM"))
        wpsp = ctx.enter_context(tc.tile_pool(name="wps", bufs=1, space="PSUM"))

        w1dr = const.tile([E, 2, H1], fp8)
        nc.scalar.dma_start(w1dr, w1dr_d[:])
        w2c = const.tile([H1, H2], bf16)
        nc.scalar.dma_start(w2c, w2_d[:])
        w3s = const.tile([2 * H2, 32], bf16)
        nc.scalar.dma_start(w3s, w3s_d[:])
        b2s = const.tile([2 * H2, 1], f32)
        nc.scalar.dma_start(b2s, b2s_d[:])
        b3t = const.tile([128, 1], f32)
        nc.scalar.dma_start(b3t, b3v_d[:].to_broadcast((128, 1)))
        mskt = const.tile([128, NCHUNK, CHUNK], bf16)
        nc.scalar.dma_start(mskt, mskt_d[:])
        ident_bf = const.tile([128, 128], bf16)
        make_identity(nc, ident_bf)
        outbuf = const.tile([E, NCHUNK, 2 * CHUNK], f32)
        nc.vector.memset(outbuf, 0.0)

        for ci in range(NCHUNK):
            W = widths[ci]
            # ---- input DMAs: alternate queues per chunk for balance ----
            rhs8 = kpool.tile([E, CHUNK, 2, W], fp8, tag=f"rhs{ci}")
            ktm = kpool.tile([128, CHUNK, E], bf16, tag=f"ktm{ci}")
            if ci % 2 == 0:
                nc.sync.dma_start(rhs8, rhs_d[ci][:])
                nc.gpsimd.dma_start(ktm[0:W], ktm_d[ci][:])
            else:
                nc.gpsimd.dma_start(rhs8, rhs_d[ci][:])
                nc.sync.dma_start(ktm[0:W], ktm_d[ci][:])

            ps3 = ps3p.tile([128, 512], f32, tag="ps3")
            pso = ps3[:, 448:512]
            h1s = []
            for t in range(4):        # 8 rows per t
                h1 = work.tile([128, 8, W], bf16, tag="h1")
                h1s.append(h1)
                ps1 = ps1p.tile([128, 2, 512], f32, tag="ps1")
                for u in range(8):
                    slot = 8 * t + u
                    nc.tensor.matmul(
                        ps1[:, u // 4, (u % 4) * W:(u % 4 + 1) * W],
                        w1dr, rhs8[:, slot, :, :],
                        start=True, stop=True, perf_mode=DR,
                        skip_group_check=True)
                src = ps1[:, :, 0:4 * W].rearrange("p a (c b) -> p a c b", b=W)
                dst = h1.rearrange("p (a c) b -> p a c b", a=2)
                if t % 2 == 0:
                    nc.scalar.activation(dst, src, AF.Relu)
                else:
                    nc.vector.tensor_scalar(dst, src, 0.0, None, ALU.max)
                # W2: 8 rows into one ps2 bank-half
                if t % 2 == 0:
                    ps2 = ps2p.tile([128, 2, 512], f32, tag="ps2")
                for u in range(8):
                    qq, r = u // 2, u % 2
                    nc.tensor.matmul(
                        ps2[64 * r:64 * r + 64, t % 2, qq * W:(qq + 1) * W],
                        w2c, h1[:, u, :], start=True, stop=True,
                        tile_position=(0, 64 * r), skip_group_check=True)
                if t % 2 == 1:
                    h2 = work.tile([128, 2, 4, W], bf16, tag="h2")
                    src2 = ps2[:, :, 0:4 * W].rearrange(
                        "p a (c b) -> p a c b", b=W)
                    if t == 1:
                        nc.vector.tensor_scalar(h2, src2, b2s[:, 0:1], 0.0,
                                                ALU.add, ALU.max)
                    else:
                        nc.scalar.activation(h2, src2, AF.Relu,
                                             bias=b2s[:, 0:1])
                    for tt in range(2):
                        j = (t - 1) + tt
                        nc.tensor.matmul(
                            ps3[32 * j:32 * j + 32, 0:4 * W], w3s,
                            h2[:, tt, :, :].rearrange("p a b -> p (a b)"),
                            start=True, stop=True, tile_position=(0, 32 * j),
                            skip_group_check=True)
            # ---- exp over all 32 rows' scores ----
            u_sp = spool.tile([98, 4, W], bf16, tag="usp")
            nc.scalar.activation(
                u_sp, ps3[0:98, 0:4 * W].rearrange("p (a b) -> p a b", b=W),
                AF.Exp, bias=b3t[0:98, 0:1])
            wps = wpsp.tile([128, 4, 128], bf16, tag="wps")
            for qq in range(4):
                nc.tensor.transpose(wps[0:W, qq, 0:98], u_sp[:, qq, 0:W],
                                    ident_bf[0:98, 0:98])
            # evac live transpose columns (rows 32j+r) to SBUF for the PE
            wcol = spool.tile([128, 4, 4, 2], bf16, tag="wcol")
            wv = wps.rearrange("p q (j x) -> p q j x", x=32)
            nc.vector.tensor_copy(wcol[0:W], wv[0:W, :, :, 0:2])
            # ---- weighted sums + U ----
            for slot in range(CHUNK):
                j, qq, r = slot // 8, (slot % 8) // 2, slot % 2
                wc = wcol[0:W, qq, j, r:r + 1]
                nc.tensor.matmul(pso[:, slot:slot + 1], ktm[0:W, slot, :], wc,
                                 start=True, stop=True, skip_group_check=True)
                nc.tensor.matmul(pso[0:1, CHUNK + slot:CHUNK + slot + 1],
                                 mskt[0:W, ci, slot:slot + 1], wc,
                                 start=True, stop=True, skip_group_check=True)
            nc.vector.tensor_copy(outbuf[:, ci, 0:CHUNK], pso[:, 0:CHUNK])
            nc.vector.tensor_copy(outbuf[0:1, ci, CHUNK:2 * CHUNK],
                                  pso[0:1, CHUNK:2 * CHUNK])
        nc.scalar.dma_start(out_d[:], outbuf.rearrange("p a b -> p (a b)"))
    nc.compile()
    return nc


def _host_prep(querys, keys, W1, b1, W2, b2, W3, b3, mask):
    import ml_dtypes
    bf = ml_dtypes.bfloat16
    f8 = ml_dtypes.float8_e4m3
    q = np.ascontiguousarray(querys[:, 0, :], dtype=np.float32)   # [B, E]
    W1a, W1b, W1c, W1d = W1[0:128], W1[128:256], W1[256:384], W1[384:512]
    W1bc = (W1b - W1c).astype(np.float32)
    qb = q @ (W1a + W1c) + b1                                      # [B, H1]
    # min-norm absorption of qb into the two DoubleRow planes
    A = np.concatenate([W1bc.T, W1d.T], axis=1)                    # [128, 256]
    av = (A.T @ np.linalg.solve(A @ A.T, qb.T)).T                  # [B, 256]
    a_b, v_b = av[:, :128], av[:, 128:]

    counts = mask.sum(axis=1).astype(np.int64)                     # [B]
    assert counts.max() <= 128, f"token count {counts.max()} > 128 unsupported"
    order = np.argsort(counts, kind="stable")                      # ascending
    widths = []
    for ci in range(NCHUNK):
        band = order[ci * NCORES * CHUNK:(ci + 1) * NCORES * CHUNK]
        widths.append(max(8, int(-(-counts[band].max() // 8) * 8)))
    widths = tuple(int(w) for w in widths)

    # row assignment: core c, chunk ci, slot s -> order[ci*256 + c*32 + s]
    assign = order.reshape(NCHUNK, NCORES, CHUNK)

    rhs_arrs = [[] for _ in range(NCORES)]
    ktm_arrs = [[] for _ in range(NCORES)]
    mskt_arr = np.zeros((NCORES, 128, NCHUNK, CHUNK), np.float32)
    for ci in range(NCHUNK):
        W = widths[ci]
        for c in range(NCORES):
            rows = assign[ci, c]                                   # [32]
            Kg = np.zeros((CHUNK, W, E), np.float32)
            for s_i, r_i in enumerate(rows):
                toks = np.nonzero(mask[r_i])[0]
                Kg[s_i, :len(toks)] = keys[r_i, toks]
                mskt_arr[c, :len(toks), ci, s_i] = 1.0
            p0 = Kg + a_b[rows][:, None, :]
            p1 = Kg * q[rows][:, None, :] + v_b[rows][:, None, :]
            rhs = np.stack([p0, p1], axis=1)                       # [32,2,W,E]
            rhs_arrs[c].append(np.ascontiguousarray(
                rhs.transpose(3, 0, 1, 2)).astype(f8))             # [E,32,2,W]
            ktm_arrs[c].append(np.ascontiguousarray(
                Kg.transpose(1, 0, 2)).astype(bf))                 # [W,32,E]

    w1dr = np.ascontiguousarray(
        np.stack([W1bc, W1d], axis=1)).astype(f8)                  # [E,2,H1]
    w3s = np.zeros((2 * H2, 32), bf)
    w3s[0:H2, 0] = W3[:, 0].astype(bf)
    w3s[H2:, 1] = W3[:, 0].astype(bf)
    b2s = np.concatenate([b2, b2]).reshape(2 * H2, 1).astype(np.float32)
    return dict(widths=widths, assign=assign, rhs=rhs_arrs, ktm=ktm_arrs,
                mskt=mskt_arr.astype(bf), w1dr=w1dr,
                w2=W2.astype(bf), w3s=w3s, b2s=b2s,
                b3v=np.asarray(b3, np.float32).reshape(1, 1))


def kernel(querys, keys, W1, b1, W2, b2, W3, b3, mask):
    from concourse.bass_utils import run_bass_kernel_spmd

    querys = np.asarray(querys, dtype=np.float32)
    keys = np.asarray(keys, dtype=np.float32)
    W1 = np.asarray(W1, dtype=np.float32)
    b1 = np.asarray(b1, dtype=np.float32)
    W2 = np.asarray(W2, dtype=np.float32)
    b2 = np.asarray(b2, dtype=np.float32)
    W3 = np.asarray(W3, dtype=np.float32)
    b3 = np.asarray(b3, dtype=np.float32)
    mask = np.asarray(mask)
    hp = _host_prep(querys, keys, W1, b1, W2, b2, W3, b3, mask)

    widths = hp["widths"]
    if widths not in _prog_cache:
        _prog_cache[widths] = _build_program(widths)
    prog = _prog_cache[widths]

    in_maps = []
    for c in range(NCORES):
        m = {f"rhs{ci}": hp["rhs"][c][ci] for ci in range(NCHUNK)}
        m.update({f"ktm{ci}": hp["ktm"][c][ci] for ci in range(NCHUNK)})
        m.update({"mskt": hp["mskt"][c], "w1dr": hp["w1dr"], "w2": hp["w2"],
                  "w3s": hp["w3s"], "b2s": hp["b2s"], "b3v": hp["b3v"]})
        in_maps.append(m)

    res = run_bass_kernel_spmd(prog, in_maps, list(range(NCORES)))
    out = np.empty((B, E), np.float32)
    assign = hp["assign"]
    for c in range(NCORES):
        o = res.results[c]["out"].reshape(E, NCHUNK, 2 * CHUNK)
        for ci in range(NCHUNK):
            sums = o[:, ci, 0:CHUNK]                  # [E, 32]
            U = o[0, ci, CHUNK:2 * CHUNK]             # [32]
            out[assign[ci, c]] = (sums / U[None, :]).T
    return out
